# revision 32
# baseline (speedup 1.0000x reference)
"""Trainium2 Bass kernel: Gauss-Newton/ADMM x-update for 3-param IR-T1 model.

Self-contained: hardcodes shapes from the problem spec.
  x, z, beta: [16, 256, 256, 3] f32   (A, B, R1 interleaved innermost)
  rho, sigma: [1] f32                 (sigma unused by the reference)
  b:          [16, 256, 256, 8] f32
  tau:        [1, 8] f32
Returns [16, 256, 256, 3] f32.

Strategy: pure data parallel over the 1M pixels -> 8 NeuronCores.
Per core: 131072 pixels laid out as [128 partitions, 1024 free], processed
in chunks of 512 pixels/partition. Active build is _build5 ("v5"),
measured 107.9-112.6us per iteration on HW across measurement
windows (vs 235.5us for the v3 baseline), with the p^2/q2^2 Schur
squares offloaded to ScalarE:

  ACT : 16 exps (E_q = Exp(-tau_q R1), E2_q = Exp(-2 tau_q R1)) into
        [128,512,8] stacks, plus a bf16 rounding copy of b.
  PE  : the q-axis weighted sums as diagonal-stationary matmuls
        accumulating in PSUM banks (p/tE/tE2/t2E2 in fp32 at 4 cyc/row,
        sb/sbE/sbtE in bf16 - they only feed the gradient; validated
        l2 7.6e-3 vs the 2e-2 gate). float32r would be 1 cyc/row but
        hard-crashes the device (NRT_EXEC_UNIT_UNRECOVERABLE).
  Pool: u = x + beta - z, Eb_q = E_q*b_q, and the e0 = sum E2_q pairwise
        tree.
  DVE : per-pixel gradient + Schur solve of the 3x3 system on the
        constant pivot c0 = 8+rho (H[0,0] is constant!), 47 TT ops.
  Chunks are software-pipelined with a one-chunk skew: DVE consumes
  chunk k-1 while ACT/Pool/PE produce chunk k. Six of the eight sums
  are read once, directly from PSUM, by the first downstream ops.

H is SPD (JTJ + rho*I with rho>0), so the det<=0 regularization branch
of the reference is dead code. tau and rho are baked as immediates;
the diag weights ship via the extra "wts" input (see make_weights).
"""

import os

import numpy as np

import concourse.bass as bass
import concourse.mybir as mybir
from concourse.tile import TileContext
from concourse.bass_utils import run_bass_kernel_spmd

F32 = mybir.dt.float32
ALU = mybir.AluOpType
ACTF = mybir.ActivationFunctionType

NB, NY, NX, NP, NQ = 16, 256, 256, 3, 8
NCORES = 8
PIX = NB * NY * NX           # 1048576
PIX_CORE = PIX // NCORES     # 131072
PARTS = 128
NFREE = PIX_CORE // PARTS    # 1024
CHUNK = 512                  # pixels per partition per chunk
NCHUNK = NFREE // CHUNK      # 2

# walrus rejects >1 semaphore wait on one instruction; Tile's final drain
# carries one wait per outstanding proc. Split the excess onto NoOps.
_MAX_WAITS = 1

LAST_RESULTS = None  # BassKernelResults of the most recent run (for test.py)


def _split_excess_waits(nc):
    for f in nc.m.functions:
        for blk in f.blocks:
            new_insts = []
            for ins in blk.instructions:
                si = getattr(ins, "sync_info", None)
                if si is not None and si.on_wait and len(si.on_wait) > _MAX_WAITS:
                    waits = list(si.on_wait)
                    extra, keep = waits[:-_MAX_WAITS], waits[-_MAX_WAITS:]
                    for idx, w in enumerate(extra):
                        new_insts.append(
                            mybir.InstNoOp(
                                name=f"{ins.name}-ws{idx}",
                                engine=ins.engine,
                                sync_info=mybir.SyncInfo(on_wait=[w], on_update=[]),
                                bass_nofuse=True,
                            )
                        )
                    si.on_wait = keep
                new_insts.append(ins)
            blk.instructions = new_insts


def _build3(tau, rho, reps=1):
    """v3: minimize blocking semaphore waits (each costs ~45us on this system).

    Rules: ScalarE (ACT) runs only Exp/Square ops that read the input tile or
    ACT's own outputs, into per-chunk-parity plane sets (no WAR stalls within
    a rep). VectorE does everything else on fixed preallocated planes —
    same-engine ordering is free. Inputs are DMA'd once up front; output
    stored once at the end.
    """
    tau = [float(t) for t in tau]
    rho = float(rho)
    c0 = 8.0 + rho  # H[0,0]

    nc = bass.Bass()
    xd = nc.declare_dram_parameter("x", [PIX_CORE, NP], F32, isOutput=False)
    zd = nc.declare_dram_parameter("z", [PIX_CORE, NP], F32, isOutput=False)
    betad = nc.declare_dram_parameter("beta", [PIX_CORE, NP], F32, isOutput=False)
    bd = nc.declare_dram_parameter("b", [PIX_CORE, NQ], F32, isOutput=False)
    yd = nc.declare_dram_parameter("y", [PIX_CORE, NP], F32, isOutput=True)

    xr = xd.rearrange("(p f) c -> p f c", p=PARTS)
    zr = zd.rearrange("(p f) c -> p f c", p=PARTS)
    betar = betad.rearrange("(p f) c -> p f c", p=PARTS)
    br = bd.rearrange("(p f) q -> p f q", p=PARTS)
    yr = yd.rearrange("(p f) c -> p f c", p=PARTS)
    chkd = None
    if reps > 1:
        chkd = nc.declare_dram_parameter("chk", [PARTS, CHUNK], F32, isOutput=True)

    v = nc.vector
    a = nc.scalar

    with TileContext(nc) as tc:
        with tc.tile_pool(name="all", bufs=1) as pool:
            def plane(name):
                return pool.tile([PARTS, CHUNK], F32, tag=name, name=name)

            # resident inputs / output
            xin_g = pool.tile([PARTS, NFREE, NP], F32, tag="xin", name="xin_g")
            zin_g = pool.tile([PARTS, NFREE, NP], F32, tag="zin", name="zin_g")
            betain_g = pool.tile([PARTS, NFREE, NP], F32, tag="betain", name="betain_g")
            bin_g = pool.tile([PARTS, NFREE, NQ], F32, tag="bin", name="bin_g")
            yout_g = pool.tile([PARTS, NFREE, NP], F32, tag="yout", name="yout_g")
            nc.sync.dma_start(xin_g[:], xr[:, :, :])
            nc.sync.dma_start(zin_g[:], zr[:, :, :])
            nc.sync.dma_start(betain_g[:], betar[:, :, :])
            nc.sync.dma_start(bin_g[:], br[:, :, :])

            # ACT-written plane sets, one per chunk parity
            E = [[plane(f"E{s}_{q}") for q in range(NQ)] for s in range(2)]
            E2 = [[plane(f"E2{s}_{q}") for q in range(NQ - 1)] for s in range(2)]

            # accumulators (DVE-private)
            SE, StE, SE2, StE2, St2E2 = (plane(n) for n in ("SE", "StE", "SE2", "StE2", "St2E2"))
            SbE, SbtE, Sb = plane("SbE"), plane("SbtE"), plane("Sb")
            # persistents
            q2, f2, e, h = plane("q2"), plane("f2"), plane("e"), plane("h")
            C00, C01, C02 = plane("C00"), plane("C01"), plane("C02")
            C11, C12, C22 = plane("C11"), plane("C12"), plane("C22")
            invdet = plane("invdet")
            w0, w1, w2 = plane("w0"), plane("w1"), plane("w2")
            g0, g1, g2 = plane("g0"), plane("g1"), plane("g2")
            # scratch (DVE-private, reused in place)
            ta, tb, tc_, td, te, tf = (plane(n) for n in ("ta", "tb", "tc", "td", "te", "tf"))
            Eb = plane("Eb")

            chk = None
            if reps > 1:
                chk = plane("chk")
                v.memset(chk[:], 0.0)

            for it in range(NCHUNK * reps):
                k = it % NCHUNK
                s = k % 2
                sl = slice(k * CHUNK, (k + 1) * CHUNK)
                xin = xin_g[:, sl, :]
                zin = zin_g[:, sl, :]
                betain = betain_g[:, sl, :]
                bin_ = bin_g[:, sl, :]
                yout = yout_g[:, sl, :]
                Av, Bv, R1v = xin[:, :, 0], xin[:, :, 1], xin[:, :, 2]
                Es, E2s = E[s], E2[s]

                # ---- ACT program: exps + squares only ----
                for q in range(NQ):
                    a.activation(Es[q][:], R1v, ACTF.Exp, scale=-tau[q])
                for q in range(1, NQ):
                    a.activation(E2s[q - 1][:], Es[q][:], ACTF.Square)

                # ---- DVE: sums over q ----
                v.tensor_reduce(Sb[:], bin_[:], mybir.AxisListType.X, ALU.add)
                v.tensor_scalar_mul(StE[:], Es[0][:], tau[0])
                # SE2/StE2/St2E2 from E2 of q>=1 plus E0^2 handled via DVE mul
                v.tensor_mul(SE2[:], Es[0][:], Es[0][:])  # E0^2
                v.tensor_scalar_mul(StE2[:], SE2[:], tau[0])
                v.tensor_scalar_mul(St2E2[:], SE2[:], tau[0] * tau[0])
                v.tensor_add(SE[:], Es[0][:], Es[1][:])
                v.tensor_mul(Eb[:], Es[0][:], bin_[:, :, 0])
                v.tensor_copy(SbE[:], Eb[:])
                v.tensor_scalar_mul(SbtE[:], Eb[:], tau[0])
                for q in range(1, NQ):
                    tq = tau[q]
                    E2q = E2s[q - 1]
                    if q > 1:
                        v.tensor_add(SE[:], SE[:], Es[q][:])
                    v.scalar_tensor_tensor(StE[:], Es[q][:], tq, StE[:], ALU.mult, ALU.add)
                    v.tensor_add(SE2[:], SE2[:], E2q[:])
                    v.scalar_tensor_tensor(StE2[:], E2q[:], tq, StE2[:], ALU.mult, ALU.add)
                    v.scalar_tensor_tensor(St2E2[:], E2q[:], tq * tq, St2E2[:], ALU.mult, ALU.add)
                    v.tensor_mul(Eb[:], Es[q][:], bin_[:, :, q])
                    v.tensor_add(SbE[:], SbE[:], Eb[:])
                    v.scalar_tensor_tensor(SbtE[:], Eb[:], tq, SbtE[:], ALU.mult, ALU.add)

                # ---- H entries ----
                v.tensor_mul(q2[:], Bv, StE[:])
                v.tensor_mul(f2[:], Bv, StE2[:])
                v.tensor_scalar_add(e[:], SE2[:], rho)
                v.tensor_mul(ta[:], Bv, Bv)           # B^2
                v.tensor_mul(tb[:], ta[:], St2E2[:])
                v.tensor_scalar_add(h[:], tb[:], rho)

                # ---- cofactors ----
                v.tensor_mul(ta[:], e[:], h[:])
                v.tensor_mul(tb[:], f2[:], f2[:])
                v.tensor_sub(C00[:], ta[:], tb[:])
                v.tensor_mul(ta[:], SE[:], h[:])
                v.tensor_mul(tb[:], f2[:], q2[:])
                v.tensor_sub(C01[:], ta[:], tb[:])
                v.tensor_mul(ta[:], SE[:], f2[:])
                v.tensor_mul(tb[:], e[:], q2[:])
                v.tensor_sub(C02[:], ta[:], tb[:])
                v.tensor_mul(ta[:], q2[:], q2[:])
                v.scalar_tensor_tensor(C11[:], h[:], c0, ta[:], ALU.mult, ALU.subtract)
                v.tensor_mul(ta[:], SE[:], q2[:])
                v.scalar_tensor_tensor(C12[:], f2[:], c0, ta[:], ALU.mult, ALU.subtract)
                v.tensor_mul(ta[:], SE[:], SE[:])
                v.scalar_tensor_tensor(C22[:], e[:], c0, ta[:], ALU.mult, ALU.subtract)

                # ---- det, 1/det ----
                v.tensor_mul(ta[:], SE[:], C01[:])
                v.scalar_tensor_tensor(tb[:], C00[:], c0, ta[:], ALU.mult, ALU.subtract)
                v.tensor_mul(ta[:], q2[:], C02[:])
                v.tensor_add(tb[:], tb[:], ta[:])
                v.reciprocal(invdet[:], tb[:])

                # ---- gradient ----
                v.tensor_sub(w0[:], betain[:, :, 0], zin[:, :, 0])
                v.tensor_sub(w1[:], betain[:, :, 1], zin[:, :, 1])
                v.tensor_sub(w2[:], betain[:, :, 2], zin[:, :, 2])

                v.tensor_add(ta[:], Av, w0[:])        # A + w0
                v.tensor_scalar_mul(ta[:], ta[:], rho)
                v.tensor_mul(tb[:], Bv, SE[:])
                v.tensor_add(tb[:], tb[:], Sb[:])
                v.scalar_tensor_tensor(ta[:], tb[:], -0.125, ta[:], ALU.mult, ALU.add)
                v.tensor_add(g0[:], ta[:], Av)

                v.tensor_add(ta[:], Bv, w1[:])
                v.tensor_scalar_mul(ta[:], ta[:], rho)
                v.tensor_mul(tb[:], Av, SE[:])
                v.tensor_mul(tc_[:], Bv, SE2[:])
                v.tensor_sub(tb[:], tb[:], tc_[:])
                v.tensor_sub(tb[:], tb[:], SbE[:])
                v.scalar_tensor_tensor(g1[:], tb[:], -0.125, ta[:], ALU.mult, ALU.add)

                v.tensor_add(ta[:], R1v, w2[:])
                v.tensor_scalar_mul(ta[:], ta[:], rho)
                v.tensor_mul(tb[:], Av, StE[:])
                v.tensor_sub(tb[:], tb[:], f2[:])
                v.tensor_sub(tb[:], tb[:], SbtE[:])
                v.tensor_mul(tb[:], Bv, tb[:])
                v.scalar_tensor_tensor(g2[:], tb[:], 0.125, ta[:], ALU.mult, ALU.add)

                # ---- d = (C/det) @ g ; out = x - d ----
                v.tensor_mul(g0[:], g0[:], invdet[:])
                v.tensor_mul(g1[:], g1[:], invdet[:])
                v.tensor_mul(g2[:], g2[:], invdet[:])

                for c, (Ca, Cb, Cc) in enumerate(
                    [(C00, C01, C02), (C01, C11, C12), (C02, C12, C22)]
                ):
                    v.tensor_mul(ta[:], Ca[:], g0[:])
                    v.tensor_mul(tb[:], Cb[:], g1[:])
                    v.tensor_add(ta[:], ta[:], tb[:])
                    v.tensor_mul(tb[:], Cc[:], g2[:])
                    v.tensor_add(ta[:], ta[:], tb[:])
                    v.tensor_sub(yout[:, :, c], xin[:, :, c], ta[:])
                    if chk is not None and c == 2:
                        v.tensor_add(chk[:], chk[:], ta[:])

                # store this chunk's output now (runs on DMA engines,
                # overlapping the next chunk's compute). Only on the final
                # rep so timing builds don't multiply store traffic.
                if it // NCHUNK == reps - 1:
                    nc.sync.dma_start(yr[:, sl, :], yout[:])

            if chk is not None:
                nc.sync.dma_start(chkd[:], chk[:])

    _split_excess_waits(nc)
    return nc


def _build(tau, rho, reps=1, phase="full"):
    """Build the per-core Bass program. tau: 8 python floats, rho: float.

    reps>1 repeats the whole computation (identical output) — used only for
    device-time measurement via wall-clock deltas. phase in
    {"full", "sums", "exps", "noact"} truncates the computation for bisection
    benchmarks (output is garbage for phase != "full").
    """
    tau = [float(t) for t in tau]
    rho = float(rho)
    c0 = 8.0 + rho  # H[0,0]

    nc = bass.Bass()
    xd = nc.declare_dram_parameter("x", [PIX_CORE, NP], F32, isOutput=False)
    zd = nc.declare_dram_parameter("z", [PIX_CORE, NP], F32, isOutput=False)
    betad = nc.declare_dram_parameter("beta", [PIX_CORE, NP], F32, isOutput=False)
    bd = nc.declare_dram_parameter("b", [PIX_CORE, NQ], F32, isOutput=False)
    yd = nc.declare_dram_parameter("y", [PIX_CORE, NP], F32, isOutput=True)

    xr = xd.rearrange("(p f) c -> p f c", p=PARTS)
    zr = zd.rearrange("(p f) c -> p f c", p=PARTS)
    betar = betad.rearrange("(p f) c -> p f c", p=PARTS)
    br = bd.rearrange("(p f) q -> p f q", p=PARTS)
    yr = yd.rearrange("(p f) c -> p f c", p=PARTS)
    # reps>1 (timing builds only): a live checksum chain defeats dead-code
    # elimination of the repeated iterations.
    chkd = None
    if reps > 1:
        chkd = nc.declare_dram_parameter("chk", [PARTS, CHUNK], F32, isOutput=True)

    v = nc.vector
    a = nc.scalar

    with TileContext(nc) as tc:
        with (
            tc.tile_pool(name="io", bufs=1) as io,
            tc.tile_pool(name="ering", bufs=3) as ering,
            tc.tile_pool(name="e2ring", bufs=2) as e2ring,
            tc.tile_pool(name="ebring", bufs=2) as ebring,
            tc.tile_pool(name="accs", bufs=2) as accs,
            tc.tile_pool(name="hphase", bufs=1) as hp,
            tc.tile_pool(name="tmp", bufs=10) as tmpp,
        ):
            chk = None
            if reps > 1:
                chk = io.tile([PARTS, CHUNK], F32, tag="chk", name="chk", bufs=1)
                nc.vector.memset(chk[:], 0.0)

            # All inputs resident in SBUF: one DMA per tensor up front,
            # one output store at the end. (Interleaving per-chunk DMAs with
            # the compute stream measured ~2-4 ms per DMA on this system.)
            xin_g = io.tile([PARTS, NFREE, NP], F32, tag="xin", name="xin_g")
            zin_g = io.tile([PARTS, NFREE, NP], F32, tag="zin", name="zin_g")
            betain_g = io.tile([PARTS, NFREE, NP], F32, tag="betain", name="betain_g")
            bin_g = io.tile([PARTS, NFREE, NQ], F32, tag="bin", name="bin_g")
            yout_g = io.tile([PARTS, NFREE, NP], F32, tag="yout", name="yout_g")
            nc.sync.dma_start(xin_g[:], xr[:, :, :])
            nc.sync.dma_start(zin_g[:], zr[:, :, :])
            nc.sync.dma_start(betain_g[:], betar[:, :, :])
            nc.sync.dma_start(bin_g[:], br[:, :, :])

            for k in range(NCHUNK * reps):
                k = k % NCHUNK
                sl = slice(k * CHUNK, (k + 1) * CHUNK)

                xin = xin_g[:, sl, :]
                zin = zin_g[:, sl, :]
                betain = betain_g[:, sl, :]
                bin_ = bin_g[:, sl, :]
                yout = yout_g[:, sl, :]

                Av = xin[:, :, 0]
                Bv = xin[:, :, 1]
                R1v = xin[:, :, 2]

                def plane(pool, tag):
                    return pool.tile([PARTS, CHUNK], F32, tag=tag, name=tag)

                yflat = yout.rearrange("p f c -> p (f c)")

                if phase == "exps":
                    chkE = plane(accs, "SE")
                    for q in range(NQ):
                        E = plane(ering, "E")
                        a.activation(E[:], R1v, ACTF.Exp, scale=-tau[q])
                        if q == 0:
                            a.copy(chkE[:], E[:])
                        else:
                            v.tensor_add(chkE[:], chkE[:], E[:])
                    v.tensor_copy(yflat[:, 0:CHUNK], chkE[:])
                    if chk is not None:
                        v.tensor_add(chk[:], chk[:], chkE[:])
                    continue

                # ---- sums over q ----
                SE = plane(accs, "SE")
                StE = plane(accs, "StE")
                SE2 = plane(accs, "SE2")
                StE2 = plane(accs, "StE2")
                St2E2 = plane(accs, "St2E2")
                SbE = plane(accs, "SbE")
                SbtE = plane(accs, "SbtE")
                Sb = plane(accs, "Sb")

                v.tensor_reduce(Sb[:], bin_[:], mybir.AxisListType.X, ALU.add)

                for q in range(NQ):
                    tq = tau[q]
                    E = plane(ering, "E")
                    a.activation(E[:], R1v, ACTF.Exp, scale=-tq)
                    Eb = plane(ebring, "Eb")
                    v.tensor_mul(Eb[:], E[:], bin_[:, :, q])
                    if q == 0:
                        a.copy(SE[:], E[:])
                        a.mul(StE[:], E[:], tq)
                        a.activation(SE2[:], E[:], ACTF.Square)
                        a.activation(StE2[:], E[:], ACTF.Square, scale=float(np.sqrt(tq)))
                        a.activation(St2E2[:], E[:], ACTF.Square, scale=tq)
                        a.copy(SbE[:], Eb[:])
                        a.mul(SbtE[:], Eb[:], tq)
                    else:
                        E2 = plane(e2ring, "E2")
                        a.activation(E2[:], E[:], ACTF.Square)
                        v.tensor_add(SE[:], SE[:], E[:])
                        v.scalar_tensor_tensor(StE[:], E[:], tq, StE[:], ALU.mult, ALU.add)
                        v.tensor_add(SE2[:], SE2[:], E2[:])
                        v.scalar_tensor_tensor(StE2[:], E2[:], tq, StE2[:], ALU.mult, ALU.add)
                        v.scalar_tensor_tensor(St2E2[:], E2[:], tq * tq, St2E2[:], ALU.mult, ALU.add)
                        v.tensor_add(SbE[:], SbE[:], Eb[:])
                        v.scalar_tensor_tensor(SbtE[:], Eb[:], tq, SbtE[:], ALU.mult, ALU.add)

                if phase == "sums":
                    sm = plane(tmpp, "tmp")
                    v.tensor_add(sm[:], SE[:], StE[:])
                    for other in (SE2, StE2, St2E2, SbE, SbtE, Sb):
                        v.tensor_add(sm[:], sm[:], other[:])
                    v.tensor_copy(yflat[:, 0:CHUNK], sm[:])
                    if chk is not None:
                        v.tensor_add(chk[:], chk[:], sm[:])
                    continue

                # ---- H entries (c0=8+rho baked) ----
                # H = [[c0, -p, q2], [-p, e, -f2], [q2, -f2, h]]
                #  p=SE, q2=B*StE, e=SE2+rho, f2=B*StE2, h=B^2*St2E2+rho
                q2 = plane(hp, "q2")
                f2 = plane(hp, "f2")
                e = plane(hp, "e")
                h = plane(hp, "h")
                v.tensor_mul(q2[:], Bv, StE[:])
                v.tensor_mul(f2[:], Bv, StE2[:])
                v.tensor_scalar_add(e[:], SE2[:], rho)
                B2 = plane(tmpp, "tmp")
                a.activation(B2[:], Bv, ACTF.Square)
                t0 = plane(tmpp, "tmp")
                v.tensor_mul(t0[:], B2[:], St2E2[:])
                v.tensor_scalar_add(h[:], t0[:], rho)

                # ---- cofactors ----
                f2sq = plane(tmpp, "tmp")
                q2sq = plane(tmpp, "tmp")
                psq = plane(tmpp, "tmp")
                a.activation(f2sq[:], f2[:], ACTF.Square)
                a.activation(q2sq[:], q2[:], ACTF.Square)
                a.activation(psq[:], SE[:], ACTF.Square)

                C00 = plane(hp, "C00")
                C01 = plane(hp, "C01")
                C02 = plane(hp, "C02")
                C11 = plane(hp, "C11")
                C12 = plane(hp, "C12")
                C22 = plane(hp, "C22")

                t1 = plane(tmpp, "tmp")
                v.tensor_mul(t1[:], e[:], h[:])
                v.tensor_sub(C00[:], t1[:], f2sq[:])

                t2 = plane(tmpp, "tmp")
                t3 = plane(tmpp, "tmp")
                v.tensor_mul(t2[:], SE[:], h[:])
                v.tensor_mul(t3[:], f2[:], q2[:])
                v.tensor_sub(C01[:], t2[:], t3[:])

                t4 = plane(tmpp, "tmp")
                t5 = plane(tmpp, "tmp")
                v.tensor_mul(t4[:], SE[:], f2[:])
                v.tensor_mul(t5[:], e[:], q2[:])
                v.tensor_sub(C02[:], t4[:], t5[:])

                v.scalar_tensor_tensor(C11[:], h[:], c0, q2sq[:], ALU.mult, ALU.subtract)
                t6 = plane(tmpp, "tmp")
                v.tensor_mul(t6[:], SE[:], q2[:])
                v.scalar_tensor_tensor(C12[:], f2[:], c0, t6[:], ALU.mult, ALU.subtract)
                v.scalar_tensor_tensor(C22[:], e[:], c0, psq[:], ALU.mult, ALU.subtract)

                # ---- det and 1/det ----
                dt1 = plane(tmpp, "tmp")
                dt2 = plane(tmpp, "tmp")
                dt3 = plane(tmpp, "tmp")
                det = plane(hp, "det")
                invdet = plane(hp, "invdet")
                v.tensor_mul(dt1[:], SE[:], C01[:])
                v.scalar_tensor_tensor(dt2[:], C00[:], c0, dt1[:], ALU.mult, ALU.subtract)
                v.tensor_mul(dt3[:], q2[:], C02[:])
                v.tensor_add(det[:], dt2[:], dt3[:])
                v.reciprocal(invdet[:], det[:])

                # ---- gradient ----
                # w_c = beta_c - z_c
                w0 = plane(hp, "w0")
                w1 = plane(hp, "w1")
                w2 = plane(hp, "w2")
                v.tensor_sub(w0[:], betain[:, :, 0], zin[:, :, 0])
                v.tensor_sub(w1[:], betain[:, :, 1], zin[:, :, 1])
                v.tensor_sub(w2[:], betain[:, :, 2], zin[:, :, 2])

                # g0 = A - (B*SE + Sb)/8 + rho*(A + w0)
                g0 = plane(hp, "g0")
                u = plane(tmpp, "tmp")
                ru = plane(tmpp, "tmp")
                v.tensor_add(u[:], Av, w0[:])
                a.mul(ru[:], u[:], rho)
                ta = plane(tmpp, "tmp")
                tb = plane(tmpp, "tmp")
                v.tensor_mul(ta[:], Bv, SE[:])
                v.tensor_add(tb[:], ta[:], Sb[:])
                g0a = plane(tmpp, "tmp")
                v.scalar_tensor_tensor(g0a[:], tb[:], -0.125, ru[:], ALU.mult, ALU.add)
                v.tensor_add(g0[:], g0a[:], Av)

                # g1 = -(A*SE - B*SE2 - SbE)/8 + rho*(B + w1)
                g1 = plane(hp, "g1")
                vb = plane(tmpp, "tmp")
                rv = plane(tmpp, "tmp")
                v.tensor_add(vb[:], Bv, w1[:])
                a.mul(rv[:], vb[:], rho)
                tc1 = plane(tmpp, "tmp")
                tc2 = plane(tmpp, "tmp")
                v.tensor_mul(tc1[:], Av, SE[:])
                v.tensor_mul(tc2[:], Bv, SE2[:])
                tc3 = plane(tmpp, "tmp")
                v.tensor_sub(tc3[:], tc1[:], tc2[:])
                tc4 = plane(tmpp, "tmp")
                v.tensor_sub(tc4[:], tc3[:], SbE[:])
                v.scalar_tensor_tensor(g1[:], tc4[:], -0.125, rv[:], ALU.mult, ALU.add)

                # g2 = B*(A*StE - B*StE2 - SbtE)/8 + rho*(R1 + w2)
                g2 = plane(hp, "g2")
                v2t = plane(tmpp, "tmp")
                rv2 = plane(tmpp, "tmp")
                v.tensor_add(v2t[:], R1v, w2[:])
                a.mul(rv2[:], v2t[:], rho)
                td1 = plane(tmpp, "tmp")
                v.tensor_mul(td1[:], Av, StE[:])
                td2 = plane(tmpp, "tmp")
                v.tensor_sub(td2[:], td1[:], f2[:])
                td3 = plane(tmpp, "tmp")
                v.tensor_sub(td3[:], td2[:], SbtE[:])
                td4 = plane(tmpp, "tmp")
                v.tensor_mul(td4[:], Bv, td3[:])
                v.scalar_tensor_tensor(g2[:], td4[:], 0.125, rv2[:], ALU.mult, ALU.add)

                # ---- d = (C/det) @ g ;  out = x - d ----
                g0s = plane(hp, "g0s")
                g1s = plane(hp, "g1s")
                g2s = plane(hp, "g2s")
                v.tensor_mul(g0s[:], g0[:], invdet[:])
                v.tensor_mul(g1s[:], g1[:], invdet[:])
                v.tensor_mul(g2s[:], g2[:], invdet[:])

                for c, (Ca, Cb, Cc) in enumerate(
                    [(C00, C01, C02), (C01, C11, C12), (C02, C12, C22)]
                ):
                    m0 = plane(tmpp, "tmp")
                    m1 = plane(tmpp, "tmp")
                    m2 = plane(tmpp, "tmp")
                    v.tensor_mul(m0[:], Ca[:], g0s[:])
                    v.tensor_mul(m1[:], Cb[:], g1s[:])
                    v.tensor_mul(m2[:], Cc[:], g2s[:])
                    s0 = plane(tmpp, "tmp")
                    v.tensor_add(s0[:], m0[:], m1[:])
                    dsum = plane(tmpp, "tmp")
                    v.tensor_add(dsum[:], s0[:], m2[:])
                    v.tensor_sub(yout[:, :, c], xin[:, :, c], dsum[:])
                    if chk is not None and c == 2:
                        v.tensor_add(chk[:], chk[:], dsum[:])

            nc.sync.dma_start(yr[:, :, :], yout_g[:])
            if chk is not None:
                nc.sync.dma_start(chkd[:], chk[:])

    _split_excess_waits(nc)
    return nc


def _build4(tau, rho, reps=1):
    """v4: PE-accumulated q-sums + Schur solve on the constant c0=8+rho pivot.

    Per chunk ([128, 512] planes, 65536 pixels):
      ACT: E_q = Exp(-tau_q R1), E2_q = Exp(-2 tau_q R1)   (16 ops)
      DVE: Eb_q = E_q * b_q                                 (8 TT)
      PE:  8 PSUM banks accumulate the 8 q-sums via diagonal fp32r matmuls
           (I, tau_q I, tau_q^2 I stationaries)             (64 MMs)
      ACT: copy p=SE, e0=SE2 PSUM->SBUF (multi-read sums); the other 6 sums
           are read directly from PSUM by early downstream ops
      DVE: gradient + Schur 2x2 solve + output               (~53 TT)
    Slots are software-pipelined with a one-chunk skew: downstream of chunk
    k overlaps ACT/PE production of chunk k+1.
    """
    tau = [float(t) for t in tau]
    rho = float(rho)
    c0 = 8.0 + rho
    F32R = mybir.dt.float32r

    nc = bass.Bass()
    xd = nc.declare_dram_parameter("x", [PIX_CORE, NP], F32, isOutput=False)
    zd = nc.declare_dram_parameter("z", [PIX_CORE, NP], F32, isOutput=False)
    betad = nc.declare_dram_parameter("beta", [PIX_CORE, NP], F32, isOutput=False)
    bd = nc.declare_dram_parameter("b", [PIX_CORE, NQ], F32, isOutput=False)
    wtsd = nc.declare_dram_parameter("wts", [PARTS, 17 * PARTS], F32, isOutput=False)
    yd = nc.declare_dram_parameter("y", [PIX_CORE, NP], F32, isOutput=True)

    xr = xd.rearrange("(p f) c -> p f c", p=PARTS)
    zr = zd.rearrange("(p f) c -> p f c", p=PARTS)
    betar = betad.rearrange("(p f) c -> p f c", p=PARTS)
    br = bd.rearrange("(p f) q -> p f q", p=PARTS)
    yr = yd.rearrange("(p f) c -> p f c", p=PARTS)
    chkd = None
    if reps > 1:
        chkd = nc.declare_dram_parameter("chk", [PARTS, CHUNK], F32, isOutput=True)

    v = nc.vector
    a = nc.scalar
    te = nc.tensor
    SUMS = ("p", "tE", "e0", "tE2", "t2E2", "sb", "sbE", "sbtE")

    with TileContext(nc) as tc:
        with (
            tc.tile_pool(name="all", bufs=1) as pool,
            tc.tile_pool(name="ps", bufs=1, space="PSUM") as pp,
        ):
            def plane(name):
                return pool.tile([PARTS, CHUNK], F32, tag=name, name=name)

            xin_g = pool.tile([PARTS, NFREE, NP], F32, tag="xin", name="xin_g")
            zin_g = pool.tile([PARTS, NFREE, NP], F32, tag="zin", name="zin_g")
            betain_g = pool.tile([PARTS, NFREE, NP], F32, tag="betain", name="betain_g")
            bin_g = pool.tile([PARTS, NFREE, NQ], F32, tag="bin", name="bin_g")
            yout_g = pool.tile([PARTS, NFREE, NP], F32, tag="yout", name="yout_g")
            wts = pool.tile([PARTS, 17 * PARTS], F32, tag="wts", name="wts")
            nc.sync.dma_start(xin_g[:], xr[:, :, :])
            nc.sync.dma_start(zin_g[:], zr[:, :, :])
            nc.sync.dma_start(betain_g[:], betar[:, :, :])
            nc.sync.dma_start(bin_g[:], br[:, :, :])
            nc.sync.dma_start(wts[:], wtsd[:, :])

            # E-sums run fp32 matmuls (accuracy: they feed the Hessian).
            # b-sums run bf16 matmuls (they only feed the gradient; bf16
            # noise there costs ~3e-3 l2, validated vs the 2e-2 gate).
            BF16 = mybir.dt.bfloat16
            wtsb = pool.tile([PARTS, 9 * PARTS], BF16, tag="wtsb", name="wtsb")
            a.copy(wtsb[:], wts[:, 0 : 9 * PARTS])
            b_r = pool.tile([PARTS, CHUNK, NQ], BF16, tag="b_r", name="b_r")

            W_I = wts[:, 0:PARTS]
            W_t = [wts[:, (1 + q) * PARTS : (2 + q) * PARTS] for q in range(NQ)]
            W_t2 = [wts[:, (9 + q) * PARTS : (10 + q) * PARTS] for q in range(NQ)]
            Wb_I = wtsb[:, 0:PARTS]
            Wb_t = [wtsb[:, (1 + q) * PARTS : (2 + q) * PARTS] for q in range(NQ)]

            Eq = [plane(f"Eq{q}") for q in range(NQ)]
            E2q = [plane(f"E2q{q}") for q in range(NQ)]
            Ebq = [
                pool.tile([PARTS, CHUNK], BF16, tag=f"Ebq{q}", name=f"Ebq{q}")
                for q in range(NQ)
            ]
            ps = {
                s: pp.tile([PARTS, CHUNK], F32, tag=f"ps_{s}", name=f"ps_{s}")
                for s in SUMS
            }
            # multi-read sums, ping-ponged in SBUF
            pS = [plane(f"pS{i}") for i in range(2)]
            e0S = [plane(f"e0S{i}") for i in range(2)]
            # downstream planes (DVE-private; same-engine ordering makes
            # reuse free). 11 long-lived + 6 rotating scratch.
            names = "q2 f2 G0 S11 S22 S12 rec gt1 gt2 d1 d2 s0 s1 s2 s3 s4 s5".split()
            P = {n: plane(n) for n in names}

            chk = None
            if reps > 1:
                chk = plane("chk")
                v.memset(chk[:], 0.0)

            nslots = NCHUNK * reps

            def produce(it):
                k = it % NCHUNK
                s = it % 2
                sl = slice(k * CHUNK, (k + 1) * CHUNK)
                R1v = xin_g[:, sl, 2]
                binc = bin_g[:, sl, :]
                a.copy(b_r[:], binc[:, :, :])
                for q in range(NQ):
                    a.activation(Eq[q][:], R1v, ACTF.Exp, scale=-tau[q])
                    a.activation(E2q[q][:], R1v, ACTF.Exp, scale=-2.0 * tau[q])
                for q in range(NQ):
                    v.tensor_mul(Ebq[q][:], Eq[q][:], binc[:, :, q])
                for q in range(NQ):
                    st, sp = (q == 0), (q == NQ - 1)
                    eq = Eq[q][:]
                    e2 = E2q[q][:]
                    eb = Ebq[q][:]
                    bq = b_r[:, :, q]
                    te.matmul(ps["p"][:], W_I, eq, start=st, stop=sp)
                    te.matmul(ps["e0"][:], W_I, e2, start=st, stop=sp)
                    te.matmul(ps["sb"][:], Wb_I, bq, start=st, stop=sp)
                    te.matmul(ps["sbE"][:], Wb_I, eb, start=st, stop=sp)
                    te.matmul(ps["tE"][:], W_t[q], eq, start=st, stop=sp)
                    te.matmul(ps["tE2"][:], W_t[q], e2, start=st, stop=sp)
                    te.matmul(ps["sbtE"][:], Wb_t[q], eb, start=st, stop=sp)
                    te.matmul(ps["t2E2"][:], W_t2[q], e2, start=st, stop=sp)
                a.copy(pS[s][:], ps["p"][:])
                a.copy(e0S[s][:], ps["e0"][:])

            def consume(it):
                k = it % NCHUNK
                s = it % 2
                sl = slice(k * CHUNK, (k + 1) * CHUNK)
                A = xin_g[:, sl, 0]
                Bv = xin_g[:, sl, 1]
                R1v = xin_g[:, sl, 2]
                p, e0 = pS[s], e0S[s]

                q2, f2, G0 = P["q2"], P["f2"], P["G0"]
                S11, S22, S12, rec = P["S11"], P["S22"], P["S12"], P["rec"]
                gt1, gt2, d1, d2 = P["gt1"], P["gt2"], P["d1"], P["d2"]
                s0, s1, s2, s3, s4, s5 = (
                    P["s0"], P["s1"], P["s2"], P["s3"], P["s4"], P["s5"]
                )

                # --- early PSUM consumers (free the banks for next produce) ---
                v.tensor_mul(q2[:], Bv, ps["tE"][:])
                v.tensor_mul(f2[:], Bv, ps["tE2"][:])
                v.tensor_mul(s0[:], Bv, Bv)                     # B^2
                v.tensor_mul(s1[:], s0[:], ps["t2E2"][:])       # hh (live to S22)
                v.tensor_mul(s0[:], Bv, p[:])                   # m1 = B*p
                v.tensor_add(s2[:], s0[:], ps["sb"][:])         # m2 (live to G0)
                v.tensor_mul(s0[:], A, p[:])                    # mA
                v.tensor_mul(s3[:], Bv, e0[:])                  # mB
                v.tensor_sub(s0[:], s0[:], s3[:])               # sG1
                v.tensor_sub(s0[:], s0[:], ps["sbE"][:])        # sG1b
                v.tensor_mul(s3[:], Bv, ps["sbtE"][:])          # mE (live to sG2b)

                # --- gradient ---
                v.tensor_sub(s4[:], betain_g[:, sl, 0], zin_g[:, sl, 0])
                v.tensor_add(s4[:], A, s4[:])                   # u0
                v.scalar_tensor_tensor(s4[:], s4[:], rho, A, ALU.mult, ALU.add)
                v.scalar_tensor_tensor(G0[:], s2[:], -0.125, s4[:], ALU.mult, ALU.add)
                v.tensor_sub(s2[:], betain_g[:, sl, 1], zin_g[:, sl, 1])
                v.tensor_add(s2[:], Bv, s2[:])                  # u1
                v.tensor_scalar_mul(s2[:], s2[:], rho)          # r1
                v.scalar_tensor_tensor(s5[:], s0[:], -0.125, s2[:], ALU.mult, ALU.add)  # G1
                v.tensor_sub(s0[:], betain_g[:, sl, 2], zin_g[:, sl, 2])
                v.tensor_add(s0[:], R1v, s0[:])                 # u2
                v.tensor_scalar_mul(s0[:], s0[:], rho)          # r2
                v.tensor_mul(s2[:], A, q2[:])                   # mC
                v.tensor_mul(s4[:], Bv, f2[:])                  # mD
                v.tensor_sub(s2[:], s2[:], s4[:])               # sG2
                v.tensor_sub(s2[:], s2[:], s3[:])               # sG2b
                v.scalar_tensor_tensor(s3[:], s2[:], 0.125, s0[:], ALU.mult, ALU.add)  # G2 (live to gt2)

                # --- Schur 2x2 system ---
                v.tensor_mul(s0[:], p[:], p[:])                 # p^2
                v.tensor_scalar_add(s2[:], e0[:], rho)          # e0 + rho
                v.scalar_tensor_tensor(S11[:], s0[:], -1.0 / c0, s2[:], ALU.mult, ALU.add)
                v.tensor_mul(s0[:], q2[:], q2[:])               # q2^2
                v.tensor_scalar_add(s2[:], s1[:], rho)          # hh + rho
                v.scalar_tensor_tensor(S22[:], s0[:], -1.0 / c0, s2[:], ALU.mult, ALU.add)
                v.tensor_mul(s0[:], p[:], q2[:])                # p*q2
                v.scalar_tensor_tensor(S12[:], s0[:], 1.0 / c0, f2[:], ALU.mult, ALU.subtract)
                v.tensor_mul(s0[:], S11[:], S22[:])
                v.tensor_mul(s1[:], S12[:], S12[:])
                v.tensor_sub(s0[:], s0[:], s1[:])               # Delta
                v.reciprocal(rec[:], s0[:])

                # --- back-substitute ---
                v.tensor_mul(s0[:], p[:], G0[:])
                v.scalar_tensor_tensor(gt1[:], s0[:], 1.0 / c0, s5[:], ALU.mult, ALU.add)
                v.tensor_mul(s0[:], q2[:], G0[:])
                v.scalar_tensor_tensor(gt2[:], s0[:], -1.0 / c0, s3[:], ALU.mult, ALU.add)
                v.tensor_mul(s0[:], S22[:], gt1[:])
                v.tensor_mul(s1[:], S12[:], gt2[:])
                v.tensor_sub(s0[:], s0[:], s1[:])               # n1
                v.tensor_mul(s2[:], S11[:], gt2[:])
                v.tensor_mul(s3[:], S12[:], gt1[:])
                v.tensor_sub(s2[:], s2[:], s3[:])               # n2
                v.tensor_mul(d1[:], s0[:], rec[:])
                v.tensor_mul(d2[:], s2[:], rec[:])
                v.tensor_mul(s0[:], p[:], d1[:])
                v.tensor_mul(s1[:], q2[:], d2[:])
                v.tensor_sub(s0[:], s0[:], s1[:])               # p*d1 - q2*d2
                v.tensor_add(s0[:], s0[:], G0[:])               # + G0
                yout = yout_g[:, sl, :]
                v.scalar_tensor_tensor(yout[:, :, 0], s0[:], -1.0 / c0, A, ALU.mult, ALU.add)
                v.scalar_tensor_tensor(yout[:, :, 1], d1[:], -1.0, Bv, ALU.mult, ALU.add)
                v.scalar_tensor_tensor(yout[:, :, 2], d2[:], -1.0, R1v, ALU.mult, ALU.add)
                if chk is not None:
                    v.tensor_add(chk[:], chk[:], d2[:])
                if it // NCHUNK == reps - 1:
                    nc.sync.dma_start(yr[:, sl, :], yout[:])

            for it in range(nslots + 1):
                if it < nslots:
                    produce(it)
                if it >= 1:
                    consume(it - 1)

            if chk is not None:
                nc.sync.dma_start(chkd[:], chk[:])

    _split_excess_waits(nc)
    return nc



def _build5(tau, rho, reps=1, phase="full"):
    """v5+v8c: engine-balanced pipeline, measured 107.9us/rep as pure v5.

    Per chunk ([128, 512] planes):
      ACT : 16 exps into E/E2 stacks, bf16 b copy, p PSUM->SBUF copy,
            psq/q2s squares for the Schur block
      Pool: u_c = x + beta - z, Eb_q = E_q*b_q (bf16 out), e0 pairwise
            tree, the final out1/out2 subtractions
      PE  : p/tE/tE2/t2E2 fp32 diag matmuls (32), sb/sbE/sbtE bf16 (24)
      DVE : ~46 TT + reciprocal downstream (Schur solve on the constant
            c0 = 8+rho pivot)
    One-chunk skew: DVE consumes chunk k-1 while ACT/Pool/PE produce k.
    tE/tE2/t2E2/sb/sbE/sbtE are read once, directly from PSUM.
    """
    tau = [float(t) for t in tau]
    rho = float(rho)
    c0 = 8.0 + rho
    BF16 = mybir.dt.bfloat16

    nc = bass.Bass()
    xd = nc.declare_dram_parameter("x", [PIX_CORE, NP], F32, isOutput=False)
    zd = nc.declare_dram_parameter("z", [PIX_CORE, NP], F32, isOutput=False)
    betad = nc.declare_dram_parameter("beta", [PIX_CORE, NP], F32, isOutput=False)
    bd = nc.declare_dram_parameter("b", [PIX_CORE, NQ], F32, isOutput=False)
    wtsd = nc.declare_dram_parameter("wts", [PARTS, 17 * PARTS], F32, isOutput=False)
    yd = nc.declare_dram_parameter("y", [PIX_CORE, NP], F32, isOutput=True)

    xr = xd.rearrange("(p f) c -> p f c", p=PARTS)
    zr = zd.rearrange("(p f) c -> p f c", p=PARTS)
    betar = betad.rearrange("(p f) c -> p f c", p=PARTS)
    br = bd.rearrange("(p f) q -> p f q", p=PARTS)
    yr = yd.rearrange("(p f) c -> p f c", p=PARTS)
    chkd = None
    if reps > 1:
        chkd = nc.declare_dram_parameter("chk", [PARTS, CHUNK], F32, isOutput=True)

    v = nc.vector
    a = nc.scalar
    g = nc.gpsimd
    te = nc.tensor
    PSUMS = ("p", "tE", "tE2", "t2E2", "sb", "sbE", "sbtE")

    with TileContext(nc) as tc:
        with (
            tc.tile_pool(name="all", bufs=1) as pool,
            tc.tile_pool(name="ps", bufs=1, space="PSUM") as pp,
        ):
            def plane(name, dt=F32):
                return pool.tile([PARTS, CHUNK], dt, tag=name, name=name)

            xin_g = pool.tile([PARTS, NFREE, NP], F32, tag="xin", name="xin_g")
            zin_g = pool.tile([PARTS, NFREE, NP], F32, tag="zin", name="zin_g")
            betain_g = pool.tile([PARTS, NFREE, NP], F32, tag="betain", name="betain_g")
            bin_g = pool.tile([PARTS, NFREE, NQ], F32, tag="bin", name="bin_g")
            yout_g = pool.tile([PARTS, NFREE, NP], F32, tag="yout", name="yout_g")
            wts = pool.tile([PARTS, 17 * PARTS], F32, tag="wts", name="wts")
            nc.sync.dma_start(xin_g[:], xr[:, :, :])
            nc.sync.dma_start(zin_g[:], zr[:, :, :])
            nc.sync.dma_start(betain_g[:], betar[:, :, :])
            nc.sync.dma_start(bin_g[:], br[:, :, :])
            nc.sync.dma_start(wts[:], wtsd[:, :])

            wtsb = pool.tile([PARTS, 9 * PARTS], BF16, tag="wtsb", name="wtsb")
            a.copy(wtsb[:], wts[:, 0 : 9 * PARTS])
            W_I = wts[:, 0:PARTS]
            W_t = [wts[:, (1 + q) * PARTS : (2 + q) * PARTS] for q in range(NQ)]
            W_t2 = [wts[:, (9 + q) * PARTS : (10 + q) * PARTS] for q in range(NQ)]
            Wb_I = wtsb[:, 0:PARTS]
            Wb_t = [wtsb[:, (1 + q) * PARTS : (2 + q) * PARTS] for q in range(NQ)]

            ES = pool.tile([PARTS, CHUNK, NQ], F32, tag="ES", name="ES")
            E2S = pool.tile([PARTS, CHUNK, NQ], F32, tag="E2S", name="E2S")
            EbS = pool.tile([PARTS, CHUNK, NQ], BF16, tag="EbS", name="EbS")
            b_r = pool.tile([PARTS, CHUNK, NQ], BF16, tag="b_r", name="b_r")
            t4 = pool.tile([PARTS, CHUNK, 4], F32, tag="t4", name="t4")

            ps = {
                s: pp.tile([PARTS, CHUNK], F32, tag=f"ps_{s}", name=f"ps_{s}")
                for s in PSUMS
            }
            pS = [plane(f"pS{i}") for i in range(2)]
            e0S = [plane(f"e0S{i}") for i in range(2)]
            uS = [[plane(f"u{c}_{i}") for c in range(3)] for i in range(2)]
            sqpair = pool.tile([PARTS, CHUNK, 2], F32, tag="sqpair", name="sqpair")
            rpair = pool.tile([PARTS, CHUNK, 2], F32, tag="rpair", name="rpair")
            Spair = pool.tile([PARTS, CHUNK, 2], F32, tag="Spair", name="Spair")
            dpair = pool.tile([PARTS, CHUNK, 2], F32, tag="dpair", name="dpair")
            names = "q2 f2 G0 S12 gt1 gt2 s0 s1 s2 s3 s5".split()
            P = {n: plane(n) for n in names}

            chk = None
            if reps > 1:
                chk = plane("chk")
                v.memset(chk[:], 0.0)

            nslots = NCHUNK * reps

            def produce(it):
                k = it % NCHUNK
                s = it % 2
                sl = slice(k * CHUNK, (k + 1) * CHUNK)
                R1v = xin_g[:, sl, 2]
                binc = bin_g[:, sl, :]
                # ACT
                a.copy(b_r[:], binc[:, :, :])
                for q in range(NQ):
                    a.activation(ES[:, :, q], R1v, ACTF.Exp, scale=-tau[q])
                    a.activation(E2S[:, :, q], R1v, ACTF.Exp, scale=-2.0 * tau[q])
                # Pool: u planes, Eb products, e0 tree
                for c in range(3):
                    g.tensor_sub(uS[s][c][:], betain_g[:, sl, c], zin_g[:, sl, c])
                    g.tensor_add(uS[s][c][:], xin_g[:, sl, c], uS[s][c][:])
                for q in range(NQ):
                    g.tensor_mul(EbS[:, :, q], ES[:, :, q], binc[:, :, q])
                g.tensor_add(t4[:], E2S[:, :, 0:4], E2S[:, :, 4:8])
                g.tensor_add(t4[:, :, 0:2], t4[:, :, 0:2], t4[:, :, 2:4])
                g.tensor_add(e0S[s][:], t4[:, :, 0], t4[:, :, 1])
                # PE
                for q in range(NQ):
                    st, sp = (q == 0), (q == NQ - 1)
                    eq = ES[:, :, q]
                    e2 = E2S[:, :, q]
                    te.matmul(ps["p"][:], W_I, eq, start=st, stop=sp)
                    te.matmul(ps["tE"][:], W_t[q], eq, start=st, stop=sp)
                    te.matmul(ps["tE2"][:], W_t[q], e2, start=st, stop=sp)
                    te.matmul(ps["t2E2"][:], W_t2[q], e2, start=st, stop=sp)
                for q in range(NQ):
                    st, sp = (q == 0), (q == NQ - 1)
                    eb = EbS[:, :, q]
                    te.matmul(ps["sb"][:], Wb_I, b_r[:, :, q], start=st, stop=sp)
                    te.matmul(ps["sbE"][:], Wb_I, eb, start=st, stop=sp)
                    te.matmul(ps["sbtE"][:], Wb_t[q], eb, start=st, stop=sp)
                # ACT: p to SBUF (multi-read)
                a.copy(pS[s][:], ps["p"][:])

            def consume(it):
                k = it % NCHUNK
                s = it % 2
                sl = slice(k * CHUNK, (k + 1) * CHUNK)
                A = xin_g[:, sl, 0]
                Bv = xin_g[:, sl, 1]
                R1v = xin_g[:, sl, 2]
                p, e0 = pS[s], e0S[s]
                u0, u1, u2 = uS[s]
                q2, f2, G0 = P["q2"], P["f2"], P["G0"]
                S12 = P["S12"]
                gt1, gt2 = P["gt1"], P["gt2"]
                S11, S22 = Spair[:, :, 0], Spair[:, :, 1]
                d1, d2 = dpair[:, :, 0], dpair[:, :, 1]
                rec = rpair[:, :, 0]   # rpair free after the Spair stt
                s0, s1, s2, s3, s5 = (
                    P["s0"], P["s1"], P["s2"], P["s3"], P["s5"]
                )
                s4 = rpair[:, :, 1]    # free until the e0r/hhr adds

                # ACT: squares (p ready at slot start; q2 lands one op in;
                # both consumed ~20 DVE ops later in the Schur block)
                a.activation(sqpair[:, :, 0], p[:], ACTF.Square)

                # early PSUM consumers
                v.tensor_mul(q2[:], Bv, ps["tE"][:])
                a.activation(sqpair[:, :, 1], q2[:], ACTF.Square)
                v.tensor_mul(f2[:], Bv, ps["tE2"][:])
                v.tensor_mul(s0[:], Bv, Bv)
                v.tensor_mul(s1[:], s0[:], ps["t2E2"][:])      # hh
                v.tensor_mul(s0[:], Bv, p[:])
                v.tensor_add(s2[:], s0[:], ps["sb"][:])        # m2
                v.tensor_mul(s0[:], A, p[:])
                v.tensor_mul(s3[:], Bv, e0[:])
                v.tensor_sub(s0[:], s0[:], s3[:])
                v.tensor_sub(s0[:], s0[:], ps["sbE"][:])       # sG1b
                v.tensor_mul(s3[:], Bv, ps["sbtE"][:])         # mE

                # gradient
                v.scalar_tensor_tensor(s4, u0[:], rho, A, ALU.mult, ALU.add)
                v.scalar_tensor_tensor(G0[:], s2[:], -0.125, s4, ALU.mult, ALU.add)
                v.tensor_scalar_mul(s2[:], u1[:], rho)
                v.scalar_tensor_tensor(s5[:], s0[:], -0.125, s2[:], ALU.mult, ALU.add)  # G1
                v.tensor_scalar_mul(s0[:], u2[:], rho)
                v.tensor_mul(s2[:], A, q2[:])
                v.tensor_mul(s4, Bv, f2[:])
                v.tensor_sub(s2[:], s2[:], s4)
                v.tensor_sub(s2[:], s2[:], s3[:])
                v.scalar_tensor_tensor(s3[:], s2[:], 0.125, s0[:], ALU.mult, ALU.add)   # G2

                # Schur 2x2 (S11 & S22 fused: same stt scalar, paired planes)
                v.tensor_scalar_add(rpair[:, :, 0], e0[:], rho)
                v.tensor_scalar_add(rpair[:, :, 1], s1[:], rho)
                v.scalar_tensor_tensor(Spair[:], sqpair[:], -1.0 / c0, rpair[:], ALU.mult, ALU.add)
                v.tensor_mul(s0[:], p[:], q2[:])
                v.scalar_tensor_tensor(S12[:], s0[:], 1.0 / c0, f2[:], ALU.mult, ALU.subtract)
                v.tensor_mul(s0[:], S11, S22)
                v.tensor_mul(s1[:], S12[:], S12[:])
                v.tensor_sub(s0[:], s0[:], s1[:])
                v.reciprocal(rec, s0[:])

                # back-substitute
                v.tensor_mul(s0[:], p[:], G0[:])
                v.scalar_tensor_tensor(gt1[:], s0[:], 1.0 / c0, s5[:], ALU.mult, ALU.add)
                v.tensor_mul(s0[:], q2[:], G0[:])
                v.scalar_tensor_tensor(gt2[:], s0[:], -1.0 / c0, s3[:], ALU.mult, ALU.add)
                v.tensor_mul(s0[:], S22, gt1[:])
                v.tensor_mul(s1[:], S12[:], gt2[:])
                v.tensor_sub(s0[:], s0[:], s1[:])
                v.tensor_mul(s2[:], S11, gt2[:])
                v.tensor_mul(s3[:], S12[:], gt1[:])
                v.tensor_sub(s2[:], s2[:], s3[:])
                v.tensor_mul(d1, s0[:], rec)
                v.tensor_mul(d2, s2[:], rec)
                v.tensor_mul(s0[:], p[:], d1)
                v.tensor_mul(s1[:], q2[:], d2)
                v.tensor_sub(s0[:], s0[:], s1[:])
                v.tensor_add(s0[:], s0[:], G0[:])
                yout = yout_g[:, sl, :]
                v.scalar_tensor_tensor(yout[:, :, 0], s0[:], -1.0 / c0, A, ALU.mult, ALU.add)
                # out1 & out2 fused: (dpair * -1) + x[:, :, 1:3]
                v.scalar_tensor_tensor(yout[:, :, 1:3], dpair[:], -1.0, xin_g[:, sl, 1:3], ALU.mult, ALU.add)
                if chk is not None:
                    v.tensor_add(chk[:], chk[:], d2)
                if it // NCHUNK == reps - 1:
                    nc.sync.dma_start(yr[:, sl, :], yout[:])

            for it in range(nslots + 1):
                if it >= 1:
                    consume(it - 1)
                if it < nslots:
                    produce(it)

            if chk is not None:
                nc.sync.dma_start(chkd[:], chk[:])

    _split_excess_waits(nc)
    return nc


def _build6(tau, rho, reps=1):
    """v6: all-bf16 PE matmuls via hi/lo decomposition.

    E_q = EH_q + EL_q with EH = bf16(E) (ACT dual-exp) and EL = bf16(E - EH)
    (Pool sub). tau split host-side into bf16 hi/lo. Each fp32-accurate sum
    becomes 2-3 bf16 matmuls (products of bf16 pairs are exact in fp32 PSUM),
    keeping ~2^-16 relative accuracy at 1 cycle/row PE throughput.
    Eb products move to DVE; e0 returns to a PSUM bank (8 banks total).
    """
    tau = [float(t) for t in tau]
    rho = float(rho)
    c0 = 8.0 + rho
    BF16 = mybir.dt.bfloat16

    nc = bass.Bass()
    xd = nc.declare_dram_parameter("x", [PIX_CORE, NP], F32, isOutput=False)
    zd = nc.declare_dram_parameter("z", [PIX_CORE, NP], F32, isOutput=False)
    betad = nc.declare_dram_parameter("beta", [PIX_CORE, NP], F32, isOutput=False)
    bd = nc.declare_dram_parameter("b", [PIX_CORE, NQ], F32, isOutput=False)
    wtsd = nc.declare_dram_parameter("wts2", [PARTS, 33 * PARTS], F32, isOutput=False)
    yd = nc.declare_dram_parameter("y", [PIX_CORE, NP], F32, isOutput=True)

    xr = xd.rearrange("(p f) c -> p f c", p=PARTS)
    zr = zd.rearrange("(p f) c -> p f c", p=PARTS)
    betar = betad.rearrange("(p f) c -> p f c", p=PARTS)
    br = bd.rearrange("(p f) q -> p f q", p=PARTS)
    yr = yd.rearrange("(p f) c -> p f c", p=PARTS)
    chkd = None
    if reps > 1:
        chkd = nc.declare_dram_parameter("chk", [PARTS, CHUNK], F32, isOutput=True)

    v = nc.vector
    a = nc.scalar
    g = nc.gpsimd
    te = nc.tensor
    PSUMS = ("p", "e0", "tE", "tE2", "t2E2", "sb", "sbE", "sbtE")

    with TileContext(nc) as tc:
        with (
            tc.tile_pool(name="all", bufs=1) as pool,
            tc.tile_pool(name="ps", bufs=1, space="PSUM") as pp,
        ):
            def plane(name, dt=F32):
                return pool.tile([PARTS, CHUNK], dt, tag=name, name=name)

            xin_g = pool.tile([PARTS, NFREE, NP], F32, tag="xin", name="xin_g")
            zin_g = pool.tile([PARTS, NFREE, NP], F32, tag="zin", name="zin_g")
            betain_g = pool.tile([PARTS, NFREE, NP], F32, tag="betain", name="betain_g")
            bin_g = pool.tile([PARTS, NFREE, NQ], F32, tag="bin", name="bin_g")
            yout_g = pool.tile([PARTS, NFREE, NP], F32, tag="yout", name="yout_g")
            wts = pool.tile([PARTS, 33 * PARTS], F32, tag="wts", name="wts")
            nc.sync.dma_start(xin_g[:], xr[:, :, :])
            nc.sync.dma_start(zin_g[:], zr[:, :, :])
            nc.sync.dma_start(betain_g[:], betar[:, :, :])
            nc.sync.dma_start(bin_g[:], br[:, :, :])
            nc.sync.dma_start(wts[:], wtsd[:, :])

            wtsb = pool.tile([PARTS, 33 * PARTS], BF16, tag="wtsb", name="wtsb")
            a.copy(wtsb[:], wts[:])
            # column blocks: 0=I, 1..8=tau_h, 9..16=tau_l, 17..24=tau2_h, 25..32=tau2_l
            def wslice(i):
                return wtsb[:, i * PARTS : (i + 1) * PARTS]
            W_I = wslice(0)
            W_th = [wslice(1 + q) for q in range(NQ)]
            W_tl = [wslice(9 + q) for q in range(NQ)]
            W_t2h = [wslice(17 + q) for q in range(NQ)]
            W_t2l = [wslice(25 + q) for q in range(NQ)]

            ES = pool.tile([PARTS, CHUNK, NQ], F32, tag="ES", name="ES")
            E2S = pool.tile([PARTS, CHUNK, NQ], F32, tag="E2S", name="E2S")
            EHS = pool.tile([PARTS, CHUNK, NQ], BF16, tag="EHS", name="EHS")
            E2HS = pool.tile([PARTS, CHUNK, NQ], BF16, tag="E2HS", name="E2HS")
            ELS = pool.tile([PARTS, CHUNK, NQ], BF16, tag="ELS", name="ELS")
            E2LS = pool.tile([PARTS, CHUNK, NQ], BF16, tag="E2LS", name="E2LS")
            EbS = pool.tile([PARTS, CHUNK, NQ], BF16, tag="EbS", name="EbS")
            b_r = pool.tile([PARTS, CHUNK, NQ], BF16, tag="b_r", name="b_r")

            ps = {
                s: pp.tile([PARTS, CHUNK], F32, tag=f"ps_{s}", name=f"ps_{s}")
                for s in PSUMS
            }
            pS = [plane(f"pS{i}") for i in range(2)]
            e0S = [plane(f"e0S{i}") for i in range(2)]
            uS = [[plane(f"u{c}_{i}") for c in range(3)] for i in range(2)]
            names = "q2 f2 G0 S11 S22 S12 rec gt1 gt2 d1 d2 s0 s1 s2 s3 s4 s5".split()
            P = {n: plane(n) for n in names}

            chk = None
            if reps > 1:
                chk = plane("chk")
                v.memset(chk[:], 0.0)

            nslots = NCHUNK * reps

            def produce(it):
                k = it % NCHUNK
                s = it % 2
                sl = slice(k * CHUNK, (k + 1) * CHUNK)
                R1v = xin_g[:, sl, 2]
                binc = bin_g[:, sl, :]
                # ACT: fp32 exps + bf16-hi exps
                a.copy(b_r[:], binc[:, :, :])
                for q in range(NQ):
                    a.activation(ES[:, :, q], R1v, ACTF.Exp, scale=-tau[q])
                    a.activation(EHS[:, :, q], R1v, ACTF.Exp, scale=-tau[q])
                    a.activation(E2S[:, :, q], R1v, ACTF.Exp, scale=-2.0 * tau[q])
                    a.activation(E2HS[:, :, q], R1v, ACTF.Exp, scale=-2.0 * tau[q])
                # Pool: u planes, lo residuals
                for c in range(3):
                    g.tensor_sub(uS[s][c][:], betain_g[:, sl, c], zin_g[:, sl, c])
                    g.tensor_add(uS[s][c][:], xin_g[:, sl, c], uS[s][c][:])
                for q in range(NQ):
                    g.tensor_sub(ELS[:, :, q], ES[:, :, q], EHS[:, :, q])
                    g.tensor_sub(E2LS[:, :, q], E2S[:, :, q], E2HS[:, :, q])
                # DVE: Eb products
                for q in range(NQ):
                    v.tensor_mul(EbS[:, :, q], ES[:, :, q], binc[:, :, q])
                # PE: all-bf16 accumulation, 16 matmuls per q
                for q in range(NQ):
                    st, sp = (q == 0), (q == NQ - 1)
                    eh, el = EHS[:, :, q], ELS[:, :, q]
                    e2h, e2l = E2HS[:, :, q], E2LS[:, :, q]
                    eb = EbS[:, :, q]
                    te.matmul(ps["p"][:], W_I, eh, start=st, stop=False)
                    te.matmul(ps["p"][:], W_I, el, start=False, stop=sp)
                    te.matmul(ps["e0"][:], W_I, e2h, start=st, stop=False)
                    te.matmul(ps["e0"][:], W_I, e2l, start=False, stop=sp)
                    te.matmul(ps["tE"][:], W_th[q], eh, start=st, stop=False)
                    te.matmul(ps["tE"][:], W_th[q], el, start=False, stop=False)
                    te.matmul(ps["tE"][:], W_tl[q], eh, start=False, stop=sp)
                    te.matmul(ps["tE2"][:], W_th[q], e2h, start=st, stop=False)
                    te.matmul(ps["tE2"][:], W_th[q], e2l, start=False, stop=False)
                    te.matmul(ps["tE2"][:], W_tl[q], e2h, start=False, stop=sp)
                    te.matmul(ps["t2E2"][:], W_t2h[q], e2h, start=st, stop=False)
                    te.matmul(ps["t2E2"][:], W_t2h[q], e2l, start=False, stop=False)
                    te.matmul(ps["t2E2"][:], W_t2l[q], e2h, start=False, stop=sp)
                    te.matmul(ps["sb"][:], W_I, b_r[:, :, q], start=st, stop=sp)
                    te.matmul(ps["sbE"][:], W_I, eb, start=st, stop=sp)
                    te.matmul(ps["sbtE"][:], W_th[q], eb, start=st, stop=sp)
                a.copy(pS[s][:], ps["p"][:])
                a.copy(e0S[s][:], ps["e0"][:])

            def consume(it):
                k = it % NCHUNK
                s = it % 2
                sl = slice(k * CHUNK, (k + 1) * CHUNK)
                A = xin_g[:, sl, 0]
                Bv = xin_g[:, sl, 1]
                R1v = xin_g[:, sl, 2]
                p, e0 = pS[s], e0S[s]
                u0, u1, u2 = uS[s]
                q2, f2, G0 = P["q2"], P["f2"], P["G0"]
                S11, S22, S12, rec = P["S11"], P["S22"], P["S12"], P["rec"]
                gt1, gt2, d1, d2 = P["gt1"], P["gt2"], P["d1"], P["d2"]
                s0, s1, s2, s3, s4, s5 = (
                    P["s0"], P["s1"], P["s2"], P["s3"], P["s4"], P["s5"]
                )

                v.tensor_mul(q2[:], Bv, ps["tE"][:])
                v.tensor_mul(f2[:], Bv, ps["tE2"][:])
                v.tensor_mul(s0[:], Bv, Bv)
                v.tensor_mul(s1[:], s0[:], ps["t2E2"][:])
                v.tensor_mul(s0[:], Bv, p[:])
                v.tensor_add(s2[:], s0[:], ps["sb"][:])
                v.tensor_mul(s0[:], A, p[:])
                v.tensor_mul(s3[:], Bv, e0[:])
                v.tensor_sub(s0[:], s0[:], s3[:])
                v.tensor_sub(s0[:], s0[:], ps["sbE"][:])
                v.tensor_mul(s3[:], Bv, ps["sbtE"][:])

                v.scalar_tensor_tensor(s4[:], u0[:], rho, A, ALU.mult, ALU.add)
                v.scalar_tensor_tensor(G0[:], s2[:], -0.125, s4[:], ALU.mult, ALU.add)
                v.tensor_scalar_mul(s2[:], u1[:], rho)
                v.scalar_tensor_tensor(s5[:], s0[:], -0.125, s2[:], ALU.mult, ALU.add)
                v.tensor_scalar_mul(s0[:], u2[:], rho)
                v.tensor_mul(s2[:], A, q2[:])
                v.tensor_mul(s4[:], Bv, f2[:])
                v.tensor_sub(s2[:], s2[:], s4[:])
                v.tensor_sub(s2[:], s2[:], s3[:])
                v.scalar_tensor_tensor(s3[:], s2[:], 0.125, s0[:], ALU.mult, ALU.add)

                v.tensor_mul(s0[:], p[:], p[:])
                v.tensor_scalar_add(s2[:], e0[:], rho)
                v.scalar_tensor_tensor(S11[:], s0[:], -1.0 / c0, s2[:], ALU.mult, ALU.add)
                v.tensor_mul(s0[:], q2[:], q2[:])
                v.tensor_scalar_add(s2[:], s1[:], rho)
                v.scalar_tensor_tensor(S22[:], s0[:], -1.0 / c0, s2[:], ALU.mult, ALU.add)
                v.tensor_mul(s0[:], p[:], q2[:])
                v.scalar_tensor_tensor(S12[:], s0[:], 1.0 / c0, f2[:], ALU.mult, ALU.subtract)
                v.tensor_mul(s0[:], S11[:], S22[:])
                v.tensor_mul(s1[:], S12[:], S12[:])
                v.tensor_sub(s0[:], s0[:], s1[:])
                v.reciprocal(rec[:], s0[:])

                v.tensor_mul(s0[:], p[:], G0[:])
                v.scalar_tensor_tensor(gt1[:], s0[:], 1.0 / c0, s5[:], ALU.mult, ALU.add)
                v.tensor_mul(s0[:], q2[:], G0[:])
                v.scalar_tensor_tensor(gt2[:], s0[:], -1.0 / c0, s3[:], ALU.mult, ALU.add)
                v.tensor_mul(s0[:], S22[:], gt1[:])
                v.tensor_mul(s1[:], S12[:], gt2[:])
                v.tensor_sub(s0[:], s0[:], s1[:])
                v.tensor_mul(s2[:], S11[:], gt2[:])
                v.tensor_mul(s3[:], S12[:], gt1[:])
                v.tensor_sub(s2[:], s2[:], s3[:])
                v.tensor_mul(d1[:], s0[:], rec[:])
                v.tensor_mul(d2[:], s2[:], rec[:])
                v.tensor_mul(s0[:], p[:], d1[:])
                v.tensor_mul(s1[:], q2[:], d2[:])
                v.tensor_sub(s0[:], s0[:], s1[:])
                v.tensor_add(s0[:], s0[:], G0[:])
                yout = yout_g[:, sl, :]
                v.scalar_tensor_tensor(yout[:, :, 0], s0[:], -1.0 / c0, A, ALU.mult, ALU.add)
                v.scalar_tensor_tensor(yout[:, :, 1], d1[:], -1.0, Bv, ALU.mult, ALU.add)
                v.scalar_tensor_tensor(yout[:, :, 2], d2[:], -1.0, R1v, ALU.mult, ALU.add)
                if chk is not None:
                    v.tensor_add(chk[:], chk[:], d2[:])
                if it // NCHUNK == reps - 1:
                    nc.sync.dma_start(yr[:, sl, :], yout[:])

            for it in range(nslots + 1):
                if it >= 1:
                    consume(it - 1)
                if it < nslots:
                    produce(it)

            if chk is not None:
                nc.sync.dma_start(chkd[:], chk[:])

    _split_excess_waits(nc)
    return nc


def make_weights2(tau):
    """Host-side weights for _build6: [128, 33*128] fp32 with bf16-split tau.

    Blocks: 0=I, 1..8=bf16hi(tau_q) I, 9..16=(tau_q - hi) I,
    17..24=bf16hi(tau_q^2) I, 25..32=(tau_q^2 - hi) I.
    All values are bf16-representable so the on-device f32->bf16 copy is exact.
    """
    def bf16(v):
        u = np.float32(v).view(np.uint32) if np.isscalar(v) else None
        arr = np.asarray(v, np.float32)
        u = arr.view(np.uint32)
        r = ((u >> 16) + ((u >> 15) & 1)).astype(np.uint32) << 16
        return r.view(np.float32)

    tau = np.asarray(tau, np.float64).reshape(NQ)
    t32 = tau.astype(np.float32)
    t2 = (tau * tau).astype(np.float32)
    th = bf16(t32)
    tl = bf16((t32 - th).astype(np.float32))
    t2h = bf16(t2)
    t2l = bf16((t2 - t2h).astype(np.float32))
    eye = np.eye(PARTS, dtype=np.float32)
    cols = [eye]
    for blk in (th, tl, t2h, t2l):
        for q in range(NQ):
            cols.append((blk[q] * eye).astype(np.float32))
    return np.ascontiguousarray(np.concatenate(cols, axis=1))

def make_weights(tau):
    """Host-side stationary weights for _build4: [128, 17*128] fp32."""
    tau = np.asarray(tau, np.float64).reshape(NQ)
    eye = np.eye(PARTS, dtype=np.float32)
    cols = [eye]
    for q in range(NQ):
        cols.append((tau[q] * eye).astype(np.float32))
    for q in range(NQ):
        cols.append((tau[q] * tau[q] * eye).astype(np.float32))
    return np.ascontiguousarray(np.concatenate(cols, axis=1))


BUILD = os.environ.get("BASS_BUILD", "v5")


def build(tau, rho, reps=1):
    if BUILD == "v3":
        return _build3(tau, rho, reps=reps)
    if BUILD == "v4":
        return _build4(tau, rho, reps=reps)
    if BUILD == "v6":
        return _build6(tau, rho, reps=reps)
    return _build5(tau, rho, reps=reps)


def make_in_maps(x, z, beta, b, tau):
    """Per-core input dicts for the current build (adds wts for v4)."""
    in_maps = []
    wts = make_weights(tau) if BUILD in ("v4", "v5") else None
    wts2 = make_weights2(tau) if BUILD == "v6" else None
    for c in range(NCORES):
        sl = slice(c * PIX_CORE, (c + 1) * PIX_CORE)
        m = {
            "x": np.ascontiguousarray(x[sl]),
            "z": np.ascontiguousarray(z[sl]),
            "beta": np.ascontiguousarray(beta[sl]),
            "b": np.ascontiguousarray(b[sl]),
        }
        if wts is not None:
            m["wts"] = wts
        if wts2 is not None:
            m["wts2"] = wts2
        in_maps.append(m)
    return in_maps


def kernel(x, z, beta, rho, sigma, b, tau):
    global LAST_RESULTS
    x = np.ascontiguousarray(np.asarray(x, dtype=np.float32).reshape(PIX, NP))
    z = np.ascontiguousarray(np.asarray(z, dtype=np.float32).reshape(PIX, NP))
    beta = np.ascontiguousarray(np.asarray(beta, dtype=np.float32).reshape(PIX, NP))
    b = np.ascontiguousarray(np.asarray(b, dtype=np.float32).reshape(PIX, NQ))
    tau_vals = np.asarray(tau, dtype=np.float32).reshape(NQ)
    rho_val = float(np.asarray(rho, dtype=np.float32).reshape(()))

    nc = build(tau_vals, rho_val)
    in_maps = make_in_maps(x, z, beta, b, tau_vals)

    res = run_bass_kernel_spmd(nc, in_maps, list(range(NCORES)))
    LAST_RESULTS = res
    y = np.concatenate([res.results[c]["y"] for c in range(NCORES)], axis=0)
    return y.reshape(NB, NY, NX, NP)

